# revision 9
# baseline (speedup 1.0000x reference)
"""NT-Xent (SimCLR) contrastive loss on 8 Trainium2 NeuronCores.

Math (reference): z = l2norm(concat(x_i, x_j)) [2B, D]; sim = z @ z.T;
loss = mean_r( log(sum_{c != r} exp(sim[r,c]/T)) - sim[r, pos(r)]/T ), T = 0.5.

Sharding: data-parallel over the 2B rows. Each core gets a column-ROLLED
copy of xt = z-input transposed [D, 2B] so that its 1024 slab rows are
always columns 0:1024 and the positive-pair partner is always columns
4096:5120 -- the SPMD program is identical on all 8 cores.

Per-core device program (v2):
  1. norms^2 via GPSIMD square + PE ones-matmul (result replicated on all
     128 partitions), 1/norm = exp(-0.5*ln(norm2)) on ACT (single table
     set, preloaded), normalize xt -> zT in fp8 e4m3, resident in SBUF as
     [128, 2, 512] DoubleRow double-chunks (contraction d = s*128 + p).
  2. slab GEMM: sim[r, c] for 1024 slab rows x 8192 cols, fp8 DoubleRow
     (K=256 per matmul, 4 accumulation steps).
  3. ACT exp(2*sim) with fused accum_out row-sums; denom = rowsum - e^2
     (the fp8 self-similarity diag is ~1 +- 4e-3; subtracting the constant
     e^2 is within ~1e-5 of the reference's masked diagonal).
  4. pos_r = zT[:, r] . zT[:, r+4096] via DVE mult + PE ones-matmul.
  5. partial = [sum_r ln(denom_r), sum_r pos_r] -> DRAM [1, 2].
Host: loss = (sum_c partial[0] - 2 * sum_c partial[1]) / (2B).
"""

import sys

sys.path.insert(0, "/opt/trn_rl_repo")

import numpy as np

B = 4096
D = 1024
TWO_B = 2 * B
N_CORES = 8
SLAB = TWO_B // N_CORES  # 1024
CW = 512                 # column tile width
NCT = TWO_B // CW        # 16 column tiles
KC = D // 128            # 8 contraction chunks
K2 = KC // 2             # 4 DoubleRow double-chunks
RB = SLAB // 128         # 8 slab row blocks
E2 = float(np.exp(2.0))

_CACHE = {}


def _act_set_id(nc):
    from concourse.hw_specs import get_activation_tables
    for i, name in enumerate(get_activation_tables(nc.m.arch)):
        if name == "natural_log_exp_and_others":
            return i
    raise RuntimeError("natural_log_exp_and_others table set not found")


def _build_program():
    import concourse.bacc as bacc
    import concourse.mybir as mybir
    import concourse.tile as tile

    dt = mybir.dt
    AF = mybir.ActivationFunctionType

    nc = bacc.Bacc("TRN2", target_bir_lowering=False, debug=False,
                   num_devices=N_CORES)
    xt_d = nc.dram_tensor("xt", [D, TWO_B], dt.float32,
                          kind="ExternalInput").ap()
    out_d = nc.dram_tensor("out", [1, 2], dt.float32,
                           kind="ExternalOutput").ap()

    with tile.TileContext(nc) as tc:
        with (
            tc.tile_pool(name="const", bufs=1) as constp,
            tc.tile_pool(name="zt", bufs=K2 * NCT) as ztp,
            tc.tile_pool(name="xa", bufs=16) as xap,
            tc.tile_pool(name="sq", bufs=4) as sqp,
            tc.tile_pool(name="lnb", bufs=2) as lnbp,
            tc.tile_pool(name="invb", bufs=3) as invbp,
            tc.tile_pool(name="pp", bufs=2) as ppp,
            tc.tile_pool(name="e", bufs=4) as ep,
            tc.tile_pool(name="sr", bufs=2) as srp,
            tc.tile_pool(name="acc", bufs=1) as accp,
            tc.tile_pool(name="nps", bufs=1, space="PSUM") as npsp,
            tc.tile_pool(name="gps", bufs=3, space="PSUM") as gpsp,
            tc.tile_pool(name="sps", bufs=1, space="PSUM") as spsp,
        ):
            # pin the one ACT table set that has Square+Ln+Exp so the
            # compiler never inserts mid-kernel table reloads (~2.7us each)
            nc.scalar.add_instruction(mybir.InstLoadActFuncSet(
                name=nc.get_next_instruction_name(),
                ins=[], outs=[], act_func_set_id=_act_set_id(nc)))

            ones_bf = constp.tile([128, 128], dt.bfloat16)
            nc.vector.memset(ones_bf[:], 1.0)
            ones_bf1 = constp.tile([128, 1], dt.bfloat16)
            nc.vector.memset(ones_bf1[:], 1.0)
            ones_f1 = constp.tile([128, 1], dt.float32)
            nc.vector.memset(ones_f1[:], 1.0)
            neg_e2 = constp.tile([128, 1], dt.float32)
            nc.vector.memset(neg_e2[:], -E2)

            S_parts = [accp.tile([128, RB], dt.float32, name=f"S_r{r}")
                       for r in range(RB)]
            loss_parts = accp.tile([128, RB], dt.float32)
            lv = accp.tile([128, 1], dt.float32)
            out_sb = accp.tile([1, 2], dt.float32)
            pos_sc = accp.tile([1, 2], dt.float32)

            # ---- phase 1: norms + normalize -> zT (fp8 e4m3, resident) ----
            # zt[k2][ct][:, s, :] holds d-chunk (2*k2 + s) of column tile ct
            zt = [[None] * NCT for _ in range(K2)]
            for ct in range(NCT):
                for k2 in range(K2):
                    t = ztp.tile([128, 2, CW], dt.float8e4, tag="zt",
                                 name=f"zt_{k2}_{ct}")
                    zt[k2][ct] = t
                nps = npsp.tile([128, CW], dt.float32, tag="nps")
                xas = []
                for k in range(KC):
                    xa = xap.tile([128, CW], dt.float32, tag="xa")
                    nc.sync.dma_start(
                        xa[:], xt_d[k * 128:(k + 1) * 128,
                                    ct * CW:(ct + 1) * CW])
                    sq = sqp.tile([128, CW], dt.bfloat16, tag="sq")
                    # split squares across GPSIMD (6/8) and DVE (2/8) to
                    # balance engine load (DVE also does the normalize mults)
                    sq_eng = nc.vector if k in (3, 7) else nc.gpsimd
                    sq_eng.tensor_mul(sq[:], xa[:], xa[:])
                    nc.tensor.matmul(nps[:], ones_bf[:], sq[:],
                                     start=(k == 0), stop=(k == KC - 1))
                    xas.append(xa)
                lnb = lnbp.tile([128, CW], dt.float32, tag="lnb")
                nc.scalar.activation(lnb[:], nps[:], AF.Ln)
                invb = invbp.tile([128, CW], dt.float32, tag="invb")
                nc.scalar.activation(invb[:], lnb[:], AF.Exp, scale=-0.5)
                for k in range(KC):
                    nc.vector.tensor_mul(zt[k // 2][ct][:, k % 2, :],
                                         xas[k][:], invb[:])

            # ---- phase 2: positive pairs  pos_r = z_r . z_{r+B} ----
            # slab cols = ct 0..1, partner cols = ct 8..9 (after roll).
            for h in range(2):
                pos_ps = spsp.tile([1, CW], dt.float32, tag="sps",
                                   name=f"pos_ps{h}")
                for k in range(KC):
                    k2, s = k // 2, k % 2
                    pp = ppp.tile([128, CW], dt.bfloat16, tag="pp")
                    nc.vector.tensor_mul(pp[:], zt[k2][h][:, s, :],
                                         zt[k2][8 + h][:, s, :])
                    nc.tensor.matmul(pos_ps[:], ones_bf1[:], pp[:],
                                     start=(k == 0), stop=(k == KC - 1))
                nc.vector.reduce_sum(pos_sc[:, h:h + 1], pos_ps[:],
                                     axis=mybir.AxisListType.X)

            # ---- phase 3: slab GEMM (fp8 DoubleRow) + exp row-sums ----
            # cg-outer so work for early column groups runs as soon as their
            # zt tiles land; per-r S tiles avoid cross-r false dependencies.
            DR = mybir.MatmulPerfMode.DoubleRow
            for cg in range(4):
                for r in range(RB):
                    lct, lo = r // 4, (r % 4) * 128
                    pss = [gpsp.tile([128, 2 * CW], dt.float32, tag="gps",
                                     name=f"gp_{cg}_{r}_{jj}")
                           for jj in range(2)]
                    for k2 in range(K2):
                        lhsT = zt[k2][lct][:, :, lo:lo + 128]
                        for j in range(4):
                            nc.tensor.matmul(
                                pss[j // 2][:, (j % 2) * CW:(j % 2 + 1) * CW],
                                lhsT, zt[k2][cg * 4 + j][:],
                                start=(k2 == 0), stop=(k2 == K2 - 1),
                                perf_mode=DR)
                    for jj in range(2):
                        e = ep.tile([128, 2 * CW], dt.bfloat16, tag="e")
                        nc.scalar.activation(
                            e[:], pss[jj][:], AF.Exp, scale=2.0,
                            accum_out=S_parts[r][:, cg * 2 + jj:
                                                 cg * 2 + jj + 1])
            for r in range(RB):
                sr = srp.tile([128, 1], dt.float32, tag="sr")
                nc.vector.reduce_sum(sr[:], S_parts[r][:],
                                     axis=mybir.AxisListType.X)
                nc.scalar.activation(loss_parts[:, r:r + 1], sr[:],
                                     AF.Ln, bias=neg_e2[:])

            # ---- phase 4: final partial sums -> out ----
            nc.vector.reduce_sum(lv[:], loss_parts[:],
                                 axis=mybir.AxisListType.X)
            fin_ps = spsp.tile([1, 1], dt.float32, tag="sps")
            nc.tensor.matmul(fin_ps[:], lv[:], ones_f1[:],
                             start=True, stop=True)
            nc.scalar.copy(out_sb[:, 0:1], fin_ps[:])
            nc.vector.tensor_add(out_sb[:, 1:2], pos_sc[:, 0:1],
                                 pos_sc[:, 1:2])
            nc.sync.dma_start(out_d, out_sb[:])

    nc.compile()
    return nc


def _get_program():
    if "nc" not in _CACHE:
        _CACHE["nc"] = _build_program()
    return _CACHE["nc"]


def make_in_maps(x_i, x_j):
    x = np.concatenate([np.asarray(x_i, np.float32),
                        np.asarray(x_j, np.float32)], axis=0)
    xt = np.ascontiguousarray(x.T)  # [D, 2B]
    in_maps = []
    for c in range(N_CORES):
        s = c * SLAB
        xt_c = np.concatenate([xt[:, s:], xt[:, :s]], axis=1)
        in_maps.append({"xt": np.ascontiguousarray(xt_c)})
    return in_maps


def combine_outputs(outs):
    tot_log = 0.0
    tot_pos = 0.0
    for o in outs:
        tot_log += float(o[0, 0])
        tot_pos += float(o[0, 1])
    return np.array((tot_log - 2.0 * tot_pos) / TWO_B, dtype=np.float32)


def kernel(x_i, x_j):
    from concourse.bass_utils import run_bass_kernel_spmd

    nc = _get_program()
    in_maps = make_in_maps(x_i, x_j)
    res = run_bass_kernel_spmd(nc, in_maps, core_ids=list(range(N_CORES)))
    return combine_outputs([res.results[c]["out"] for c in range(N_CORES)])


# revision 17
# speedup vs baseline: 1.3143x; 1.3143x over previous
"""NT-Xent (SimCLR) contrastive loss on 8 Trainium2 NeuronCores.

Math (reference): z = l2norm(concat(x_i, x_j)) [2B, D]; sim = z @ z.T;
loss = mean_r( log(sum_{c != r} exp(sim[r,c]/T)) - sim[r, pos(r)]/T ), T = 0.5.

Sharding: data-parallel over the 2B rows. Each core gets a column-ROLLED
copy of xt = z-input transposed [D, 2B] so that its 1024 slab rows are
always columns 0:1024 and the positive-pair partner is always columns
4096:5120 -- the SPMD program is identical on all 8 cores.

Per-core device program (v2):
  1. norms^2 via GPSIMD square + PE ones-matmul (result replicated on all
     128 partitions), 1/norm = exp(-0.5*ln(norm2)) on ACT (single table
     set, preloaded), normalize xt -> zT in fp8 e4m3, resident in SBUF as
     [128, 2, 512] DoubleRow double-chunks (contraction d = s*128 + p).
  2. slab GEMM: sim[r, c] for 1024 slab rows x 8192 cols, fp8 DoubleRow
     (K=256 per matmul, 4 accumulation steps).
  3. ACT exp(2*sim) with fused accum_out row-sums; denom = rowsum - e^2
     (the fp8 self-similarity diag is ~1 +- 4e-3; subtracting the constant
     e^2 is within ~1e-5 of the reference's masked diagonal).
  4. pos_r = zT[:, r] . zT[:, r+4096] via DVE mult + PE ones-matmul.
  5. partial = [sum_r ln(denom_r), sum_r pos_r] -> DRAM [1, 2].
Host: loss = (sum_c partial[0] - 2 * sum_c partial[1]) / (2B).
"""

import sys

sys.path.insert(0, "/opt/trn_rl_repo")

import numpy as np

B = 4096
D = 1024
TWO_B = 2 * B
N_CORES = 8
SLAB = TWO_B // N_CORES  # 1024
CW = 512                 # column tile width
NCT = TWO_B // CW        # 16 column tiles
KC = D // 128            # 8 contraction chunks
K2 = KC // 2             # 4 DoubleRow double-chunks
RB = SLAB // 128         # 8 slab row blocks
E2 = float(np.exp(2.0))

_CACHE = {}


def _act_set_id(nc):
    from concourse.hw_specs import get_activation_tables
    for i, name in enumerate(get_activation_tables(nc.m.arch)):
        if name == "natural_log_exp_and_others":
            return i
    raise RuntimeError("natural_log_exp_and_others table set not found")


def _build_program():
    import concourse.bacc as bacc
    import concourse.mybir as mybir
    import concourse.tile as tile

    dt = mybir.dt
    AF = mybir.ActivationFunctionType

    nc = bacc.Bacc("TRN2", target_bir_lowering=False, debug=False,
                   num_devices=N_CORES)
    xt_d = nc.dram_tensor("xt", [D, TWO_B], dt.float32,
                          kind="ExternalInput").ap()
    out_d = nc.dram_tensor("out", [1, 2], dt.float32,
                           kind="ExternalOutput").ap()

    with tile.TileContext(nc) as tc:
        with (
            tc.tile_pool(name="const", bufs=1) as constp,
            tc.tile_pool(name="zt", bufs=K2 * NCT) as ztp,
            tc.tile_pool(name="xa", bufs=10) as xap,
            tc.tile_pool(name="sq", bufs=6) as sqp,
            tc.tile_pool(name="lnb", bufs=4) as lnbp,
            tc.tile_pool(name="invb", bufs=6) as invbp,
            tc.tile_pool(name="pp", bufs=4) as ppp,
            tc.tile_pool(name="e", bufs=6) as ep,
            tc.tile_pool(name="sr", bufs=4) as srp,
            tc.tile_pool(name="acc", bufs=1) as accp,
            tc.tile_pool(name="nps", bufs=1, space="PSUM") as npsp,
            tc.tile_pool(name="gps", bufs=3, space="PSUM") as gpsp,
            tc.tile_pool(name="sps", bufs=1, space="PSUM") as spsp,
        ):
            # pin the one ACT table set that has Square+Ln+Exp so the
            # compiler never inserts mid-kernel table reloads (~2.7us each)
            nc.scalar.add_instruction(mybir.InstLoadActFuncSet(
                name=nc.get_next_instruction_name(),
                ins=[], outs=[], act_func_set_id=_act_set_id(nc)))

            ones_bf = constp.tile([128, 128], dt.bfloat16)
            nc.vector.memset(ones_bf[:], 1.0)
            ones_bf1 = constp.tile([128, 1], dt.bfloat16)
            nc.vector.memset(ones_bf1[:], 1.0)
            ones_f1 = constp.tile([128, 1], dt.float32)
            nc.vector.memset(ones_f1[:], 1.0)
            neg_e2 = constp.tile([128, 1], dt.float32)
            nc.vector.memset(neg_e2[:], -E2)

            S_parts = [accp.tile([128, RB], dt.float32, name=f"S_r{r}")
                       for r in range(RB)]
            loss_parts = accp.tile([128, RB], dt.float32)
            lv = accp.tile([128, 1], dt.float32)
            out_sb = accp.tile([1, 2], dt.float32)
            pos_sc = accp.tile([1, 2], dt.float32)

            # ---- software-pipelined emission ----
            # stage A(ct): DMA + squares + norm matmuls
            # stage B(ct): ln -> 1/norm -> normalize mults (one ct behind A,
            #              so DVE/ACT never head-of-line block on the chain)
            # GEMM column-group cg is emitted right after B(4*cg+3) so PE's
            # FIFO interleaves GEMM matmuls with later norm matmuls; the pos
            # computation is emitted after B(9) (needs ct 0,1,8,9).
            DR = mybir.MatmulPerfMode.DoubleRow
            zt = [[None] * NCT for _ in range(K2)]
            xa_of = {}
            nps_of = {}

            def stage_a(ct):
                for k2 in range(K2):
                    t = ztp.tile([128, 2, CW], dt.float8e4, tag="zt",
                                 name=f"zt_{k2}_{ct}")
                    zt[k2][ct] = t
                nps = npsp.tile([128, CW], dt.float32, tag="nps",
                                name=f"nps{ct}")
                nps_of[ct] = nps
                xas = []
                for k2 in range(K2):
                    # one DMA per DoubleRow double-chunk: [128, 2, 512] with
                    # contraction d = 256*k2 + s*128 + p
                    xa = xap.tile([128, 2, CW], dt.float32, tag="xa",
                                  name=f"xa_{ct}_{k2}")
                    src = xt_d[k2 * 256:(k2 + 1) * 256,
                               ct * CW:(ct + 1) * CW]
                    nc.sync.dma_start(xa[:],
                                      src.rearrange("(s p) c -> p s c", p=128))
                    sq = sqp.tile([128, 2, CW], dt.bfloat16, tag="sq",
                                  name=f"sq_{ct}_{k2}")
                    # split squares across GPSIMD (3/4) and DVE (1/4) to
                    # balance engine load (DVE also does the normalize mults)
                    sq_eng = nc.vector if k2 == 0 else nc.gpsimd
                    sq_eng.tensor_mul(sq[:], xa[:], xa[:])
                    for s in range(2):
                        nc.tensor.matmul(nps[:], ones_bf[:], sq[:, s, :],
                                         start=(k2 == 0 and s == 0),
                                         stop=(k2 == K2 - 1 and s == 1))
                    xas.append(xa)
                xa_of[ct] = xas

            def stage_b(ct):
                lnb = lnbp.tile([128, CW], dt.float32, tag="lnb",
                                name=f"lnb{ct}")
                nc.scalar.activation(lnb[:], nps_of[ct][:], AF.Ln)
                invb = invbp.tile([128, CW], dt.float32, tag="invb",
                                  name=f"invb{ct}")
                nc.scalar.activation(invb[:], lnb[:], AF.Exp, scale=-0.5)
                invb_b = invb[:].unsqueeze(1).broadcast_to([128, 2, CW])
                for k2 in range(K2):
                    nc.vector.tensor_mul(zt[k2][ct][:], xa_of[ct][k2][:],
                                         invb_b)
                del xa_of[ct]

            def emit_pos():
                # pos_r = z_r . z_{r+B}; slab = ct 0..1, partner = ct 8..9
                for h in range(2):
                    pos_ps = spsp.tile([1, CW], dt.float32, tag="sps",
                                       name=f"pos_ps{h}")
                    for k in range(KC):
                        k2, s = k // 2, k % 2
                        pp = ppp.tile([128, CW], dt.bfloat16, tag="pp",
                                      name=f"pp_{h}_{k}")
                        nc.vector.tensor_mul(pp[:], zt[k2][h][:, s, :],
                                             zt[k2][8 + h][:, s, :])
                        nc.tensor.matmul(pos_ps[:], ones_bf1[:], pp[:],
                                         start=(k == 0), stop=(k == KC - 1))
                    nc.vector.reduce_sum(pos_sc[:, h:h + 1], pos_ps[:],
                                         axis=mybir.AxisListType.X)

            def emit_gemm_pair(pg):
                # one ct-pair (cols pg*1024 .. +1024) for all 8 row blocks
                for r in range(RB):
                    lct, lo = r // 4, (r % 4) * 128
                    ps = gpsp.tile([128, 2 * CW], dt.float32, tag="gps",
                                   name=f"gp_{pg}_{r}")
                    for k2 in range(K2):
                        lhsT = zt[k2][lct][:, :, lo:lo + 128]
                        for j in range(2):
                            nc.tensor.matmul(
                                ps[:, j * CW:(j + 1) * CW],
                                lhsT, zt[k2][pg * 2 + j][:],
                                start=(k2 == 0), stop=(k2 == K2 - 1),
                                perf_mode=DR)
                    e = ep.tile([128, 2 * CW], dt.bfloat16, tag="e",
                                name=f"e_{pg}_{r}")
                    nc.scalar.activation(
                        e[:], ps[:], AF.Exp, scale=2.0,
                        accum_out=S_parts[r][:, pg:pg + 1])

            for ct in range(NCT + 1):
                if ct < NCT:
                    stage_a(ct)
                if ct >= 1:
                    stage_b(ct - 1)
            emit_pos()
            for pg in range(RB):
                emit_gemm_pair(pg)

            for r in range(RB):
                sr = srp.tile([128, 1], dt.float32, tag="sr")
                nc.vector.reduce_sum(sr[:], S_parts[r][:],
                                     axis=mybir.AxisListType.X)
                nc.scalar.activation(loss_parts[:, r:r + 1], sr[:],
                                     AF.Ln, bias=neg_e2[:])

            # ---- phase 4: final partial sums -> out ----
            nc.vector.reduce_sum(lv[:], loss_parts[:],
                                 axis=mybir.AxisListType.X)
            fin_ps = spsp.tile([1, 1], dt.float32, tag="sps")
            nc.tensor.matmul(fin_ps[:], lv[:], ones_f1[:],
                             start=True, stop=True)
            nc.scalar.copy(out_sb[:, 0:1], fin_ps[:])
            nc.vector.tensor_add(out_sb[:, 1:2], pos_sc[:, 0:1],
                                 pos_sc[:, 1:2])
            nc.sync.dma_start(out_d, out_sb[:])

    nc.compile()
    return nc


def _get_program():
    if "nc" not in _CACHE:
        _CACHE["nc"] = _build_program()
    return _CACHE["nc"]


def make_in_maps(x_i, x_j):
    x = np.concatenate([np.asarray(x_i, np.float32),
                        np.asarray(x_j, np.float32)], axis=0)
    xt = np.ascontiguousarray(x.T)  # [D, 2B]
    in_maps = []
    for c in range(N_CORES):
        s = c * SLAB
        xt_c = np.concatenate([xt[:, s:], xt[:, :s]], axis=1)
        in_maps.append({"xt": np.ascontiguousarray(xt_c)})
    return in_maps


def combine_outputs(outs):
    tot_log = 0.0
    tot_pos = 0.0
    for o in outs:
        tot_log += float(o[0, 0])
        tot_pos += float(o[0, 1])
    return np.array((tot_log - 2.0 * tot_pos) / TWO_B, dtype=np.float32)


def kernel(x_i, x_j):
    from concourse.bass_utils import run_bass_kernel_spmd

    nc = _get_program()
    in_maps = make_in_maps(x_i, x_j)
    res = run_bass_kernel_spmd(nc, in_maps, core_ids=list(range(N_CORES)))
    return combine_outputs([res.results[c]["out"] for c in range(N_CORES)])


# revision 19
# speedup vs baseline: 1.3310x; 1.0128x over previous
"""NT-Xent (SimCLR) contrastive loss on 8 Trainium2 NeuronCores.

Math (reference): z = l2norm(concat(x_i, x_j)) [2B, D]; sim = z @ z.T;
loss = mean_r( log(sum_{c != r} exp(sim[r,c]/T)) - sim[r, pos(r)]/T ), T = 0.5.

Sharding: data-parallel over the 2B rows. Each core gets a column-ROLLED
copy of xt = z-input transposed [D, 2B] so that its 1024 slab rows are
always columns 0:1024 and the positive-pair partner is always columns
4096:5120 -- the SPMD program is identical on all 8 cores.

Per-core device program:
  1. norms^2 via squares (split GPSIMD 3/4, DVE 1/4) + PE ones-matmul
     (result replicated on all 128 partitions), 1/norm = exp(-0.5*ln(n2))
     on ACT (single table set, preloaded), normalize xt -> zT in fp8 e4m3,
     resident in SBUF as [128, 2, 512] DoubleRow double-chunks
     (contraction d = 256*k2 + s*128 + p). Emission is software-pipelined:
     stage A(ct) = DMA+square+norm-matmul, stage B(ct-1) = ln/inv/mults,
     so no engine FIFO head-of-line blocks the ln->inv->mult chain.
  2. slab GEMM: sim[r, c] for 1024 slab rows x 8192 cols, fp8 DoubleRow
     (K=256 per matmul, 4 accumulation steps), emitted in ct-pair groups
     so the post-GEMM exp tail after the last zt tile is short.
  3. ACT exp(2*sim) with fused accum_out row-sums; denom = rowsum - e^2
     (the fp8 self-similarity diag is ~1 +- 4e-3; subtracting the constant
     e^2 is within ~1e-5 of the reference's masked diagonal).
  4. pos_r = zT[:, r] . zT[:, r+4096] via DVE mult + PE ones-matmul.
  5. partial = [sum_r ln(denom_r), sum_r pos_r] -> DRAM [1, 2].
Host: loss = (sum_c partial[0] - 2 * sum_c partial[1]) / (2B).
"""

import sys

sys.path.insert(0, "/opt/trn_rl_repo")

import numpy as np

B = 4096
D = 1024
TWO_B = 2 * B
N_CORES = 8
SLAB = TWO_B // N_CORES  # 1024
CW = 512                 # column tile width
NCT = TWO_B // CW        # 16 column tiles
KC = D // 128            # 8 contraction chunks
K2 = KC // 2             # 4 DoubleRow double-chunks
RB = SLAB // 128         # 8 slab row blocks
E2 = float(np.exp(2.0))

_CACHE = {}


def _act_set_id(nc):
    from concourse.hw_specs import get_activation_tables
    for i, name in enumerate(get_activation_tables(nc.m.arch)):
        if name == "natural_log_exp_and_others":
            return i
    raise RuntimeError("natural_log_exp_and_others table set not found")


def _build_program():
    import concourse.bacc as bacc
    import concourse.mybir as mybir
    import concourse.tile as tile

    dt = mybir.dt
    AF = mybir.ActivationFunctionType

    nc = bacc.Bacc("TRN2", target_bir_lowering=False, debug=False,
                   num_devices=N_CORES)
    xt_d = nc.dram_tensor("xt", [D, TWO_B], dt.float32,
                          kind="ExternalInput").ap()
    out_d = nc.dram_tensor("out", [1, 2], dt.float32,
                           kind="ExternalOutput").ap()

    with tile.TileContext(nc) as tc:
        with (
            tc.tile_pool(name="const", bufs=1) as constp,
            tc.tile_pool(name="zt", bufs=K2 * NCT) as ztp,
            tc.tile_pool(name="xa", bufs=12) as xap,
            tc.tile_pool(name="sq", bufs=8) as sqp,
            tc.tile_pool(name="lnb", bufs=4) as lnbp,
            tc.tile_pool(name="invb", bufs=6) as invbp,
            tc.tile_pool(name="pp", bufs=4) as ppp,
            tc.tile_pool(name="e", bufs=8) as ep,
            tc.tile_pool(name="sr", bufs=4) as srp,
            tc.tile_pool(name="acc", bufs=1) as accp,
            tc.tile_pool(name="nps", bufs=1, space="PSUM") as npsp,
            tc.tile_pool(name="gps", bufs=3, space="PSUM") as gpsp,
            tc.tile_pool(name="sps", bufs=1, space="PSUM") as spsp,
        ):
            # pin the one ACT table set that has Square+Ln+Exp so the
            # compiler never inserts mid-kernel table reloads (~2.7us each)
            nc.scalar.add_instruction(mybir.InstLoadActFuncSet(
                name=nc.get_next_instruction_name(),
                ins=[], outs=[], act_func_set_id=_act_set_id(nc)))

            ones_bf = constp.tile([128, 128], dt.bfloat16)
            nc.vector.memset(ones_bf[:], 1.0)
            ones_bf1 = constp.tile([128, 1], dt.bfloat16)
            nc.vector.memset(ones_bf1[:], 1.0)
            ones_f1 = constp.tile([128, 1], dt.float32)
            nc.vector.memset(ones_f1[:], 1.0)
            neg_e2 = constp.tile([128, 1], dt.float32)
            nc.vector.memset(neg_e2[:], -E2)

            S_parts = [accp.tile([128, RB], dt.float32, name=f"S_r{r}")
                       for r in range(RB)]
            loss_parts = accp.tile([128, RB], dt.float32)
            lv = accp.tile([128, 1], dt.float32)
            out_sb = accp.tile([1, 2], dt.float32)
            pos_sc = accp.tile([1, 2], dt.float32)

            # ---- software-pipelined emission ----
            # stage A(ct): DMA + squares + norm matmuls
            # stage B(ct): ln -> 1/norm -> normalize mults (one ct behind A,
            #              so DVE/ACT never head-of-line block on the chain)
            # GEMM column-group cg is emitted right after B(4*cg+3) so PE's
            # FIFO interleaves GEMM matmuls with later norm matmuls; the pos
            # computation is emitted after B(9) (needs ct 0,1,8,9).
            DR = mybir.MatmulPerfMode.DoubleRow
            zt = [[None] * NCT for _ in range(K2)]
            xa_of = {}
            nps_of = {}

            def stage_a(ct):
                for k2 in range(K2):
                    t = ztp.tile([128, 2, CW], dt.float8e4, tag="zt",
                                 name=f"zt_{k2}_{ct}")
                    zt[k2][ct] = t
                nps = npsp.tile([128, CW], dt.float32, tag="nps",
                                name=f"nps{ct}")
                nps_of[ct] = nps
                xas = []
                for k2 in range(K2):
                    # one DMA per DoubleRow double-chunk: [128, 2, 512] with
                    # contraction d = 256*k2 + s*128 + p
                    xa = xap.tile([128, 2, CW], dt.float32, tag="xa",
                                  name=f"xa_{ct}_{k2}")
                    src = xt_d[k2 * 256:(k2 + 1) * 256,
                               ct * CW:(ct + 1) * CW]
                    nc.sync.dma_start(xa[:],
                                      src.rearrange("(s p) c -> p s c", p=128))
                    sq = sqp.tile([128, 2, CW], dt.bfloat16, tag="sq",
                                  name=f"sq_{ct}_{k2}")
                    # split squares across GPSIMD (3/4) and DVE (1/4) to
                    # balance engine load (DVE also does the normalize mults)
                    sq_eng = nc.vector if k2 == 0 else nc.gpsimd
                    sq_eng.tensor_mul(sq[:], xa[:], xa[:])
                    for s in range(2):
                        nc.tensor.matmul(nps[:], ones_bf[:], sq[:, s, :],
                                         start=(k2 == 0 and s == 0),
                                         stop=(k2 == K2 - 1 and s == 1))
                    xas.append(xa)
                xa_of[ct] = xas

            def stage_b(ct):
                lnb = lnbp.tile([128, CW], dt.float32, tag="lnb",
                                name=f"lnb{ct}")
                nc.scalar.activation(lnb[:], nps_of[ct][:], AF.Ln)
                invb = invbp.tile([128, CW], dt.float32, tag="invb",
                                  name=f"invb{ct}")
                nc.scalar.activation(invb[:], lnb[:], AF.Exp, scale=-0.5)
                invb_b = invb[:].unsqueeze(1).broadcast_to([128, 2, CW])
                for k2 in range(K2):
                    nc.vector.tensor_mul(zt[k2][ct][:], xa_of[ct][k2][:],
                                         invb_b)
                del xa_of[ct]

            def emit_pos():
                # pos_r = z_r . z_{r+B}; slab = ct 0..1, partner = ct 8..9
                for h in range(2):
                    pos_ps = spsp.tile([1, CW], dt.float32, tag="sps",
                                       name=f"pos_ps{h}")
                    for k in range(KC):
                        k2, s = k // 2, k % 2
                        pp = ppp.tile([128, CW], dt.bfloat16, tag="pp",
                                      name=f"pp_{h}_{k}")
                        nc.vector.tensor_mul(pp[:], zt[k2][h][:, s, :],
                                             zt[k2][8 + h][:, s, :])
                        nc.tensor.matmul(pos_ps[:], ones_bf1[:], pp[:],
                                         start=(k == 0), stop=(k == KC - 1))
                    nc.vector.reduce_sum(pos_sc[:, h:h + 1], pos_ps[:],
                                         axis=mybir.AxisListType.X)

            def emit_gemm_pair(pg):
                # one ct-pair (cols pg*1024 .. +1024) for all 8 row blocks
                for r in range(RB):
                    lct, lo = r // 4, (r % 4) * 128
                    ps = gpsp.tile([128, 2 * CW], dt.float32, tag="gps",
                                   name=f"gp_{pg}_{r}")
                    for k2 in range(K2):
                        lhsT = zt[k2][lct][:, :, lo:lo + 128]
                        for j in range(2):
                            nc.tensor.matmul(
                                ps[:, j * CW:(j + 1) * CW],
                                lhsT, zt[k2][pg * 2 + j][:],
                                start=(k2 == 0), stop=(k2 == K2 - 1),
                                perf_mode=DR)
                    e = ep.tile([128, 2 * CW], dt.bfloat16, tag="e",
                                name=f"e_{pg}_{r}")
                    nc.scalar.activation(
                        e[:], ps[:], AF.Exp, scale=2.0,
                        accum_out=S_parts[r][:, pg:pg + 1])

            for ct in range(NCT + 1):
                if ct < NCT:
                    stage_a(ct)
                if ct >= 1:
                    stage_b(ct - 1)
            emit_pos()
            for pg in range(RB):
                emit_gemm_pair(pg)

            for r in range(RB):
                sr = srp.tile([128, 1], dt.float32, tag="sr")
                nc.vector.reduce_sum(sr[:], S_parts[r][:],
                                     axis=mybir.AxisListType.X)
                nc.scalar.activation(loss_parts[:, r:r + 1], sr[:],
                                     AF.Ln, bias=neg_e2[:])

            # ---- phase 4: final partial sums -> out ----
            nc.vector.reduce_sum(lv[:], loss_parts[:],
                                 axis=mybir.AxisListType.X)
            fin_ps = spsp.tile([1, 1], dt.float32, tag="sps")
            nc.tensor.matmul(fin_ps[:], lv[:], ones_f1[:],
                             start=True, stop=True)
            nc.scalar.copy(out_sb[:, 0:1], fin_ps[:])
            nc.vector.tensor_add(out_sb[:, 1:2], pos_sc[:, 0:1],
                                 pos_sc[:, 1:2])
            nc.sync.dma_start(out_d, out_sb[:])

    nc.compile()
    return nc


def _get_program():
    if "nc" not in _CACHE:
        _CACHE["nc"] = _build_program()
    return _CACHE["nc"]


def make_in_maps(x_i, x_j):
    x = np.concatenate([np.asarray(x_i, np.float32),
                        np.asarray(x_j, np.float32)], axis=0)
    xt = np.ascontiguousarray(x.T)  # [D, 2B]
    in_maps = []
    for c in range(N_CORES):
        s = c * SLAB
        xt_c = np.concatenate([xt[:, s:], xt[:, :s]], axis=1)
        in_maps.append({"xt": np.ascontiguousarray(xt_c)})
    return in_maps


def combine_outputs(outs):
    tot_log = 0.0
    tot_pos = 0.0
    for o in outs:
        tot_log += float(o[0, 0])
        tot_pos += float(o[0, 1])
    return np.array((tot_log - 2.0 * tot_pos) / TWO_B, dtype=np.float32)


def kernel(x_i, x_j):
    from concourse.bass_utils import run_bass_kernel_spmd

    nc = _get_program()
    in_maps = make_in_maps(x_i, x_j)
    res = run_bass_kernel_spmd(nc, in_maps, core_ids=list(range(N_CORES)))
    return combine_outputs([res.results[c]["out"] for c in range(N_CORES)])
